# revision 9
# baseline (speedup 1.0000x reference)
"""Embedding lookup (gather rows of W.T by index, + bias) on 8 TRN2 cores.

Strategy: vocab-sharded ("row-parallel") embedding in bf16. The bias is
folded into the table on the host and the table is cast to bf16 (worst-case
elementwise relative error 2^-9 ~ 0.2%, well under the 2e-2 gate); every
device byte moved is thereby halved vs f32. Each core owns a 12500-row shard
of the 100000-row table; the host routes each token to its owning core via
one argsort (grouping by shard AND sorting ascending within it), the device
does the data movement, and the host applies the inverse permutation (and
upcast) to assemble the full [4096, 200, 64] f32 output.

Device kernel (SPMD on 8 cores, identical program), built around the gpsimd
dma_gather primitive (SWDGE: one DMA descriptor per index):

- bf16 rows are 128 B but SWDGE strides must be 256 B-aligned, so the table
  is viewed as 6250 UNITS of 2 rows (256 B). A BLOCK descriptor serves
  BLK=32 consecutive sorted tokens from a W=16-unit (32-row) window: one
  4 KB descriptor at SDMA line rate per 32 tokens. Sorted multiplicity is
  ~8.2 tokens/row, so a 32-token span covers ~4 distinct rows - the 32-row
  window essentially never misses (violators fall to the SINGLES pass).
- SINGLES pass: rare tokens outside their block window are gathered one
  256 B unit each; the host slices the right half. Unused singles slots are
  -1 (trailing negatives are skipped by the ucode), so the pass is ~free
  when empty; one valid slot is kept so the completion semaphore fires.
- 7 block chunks (<=512 descriptors each) with DEDICATED SBUF buffers - no
  buffer rotation, so gathers never wait on writes. Gathers alternate SWDGE
  queues 2/3; the two HWDGE engines (sync/scalar) stream completed buffers
  to HBM, overlapped with subsequent gathers. First chunk is small (256) to
  prime the write stream early.
- Startup: the idx loads are issued from the sync engine (up ~3 us before
  gpsimd is ready), and a dummy 16-descriptor gather warms the DMAGatherAnt
  ucode while they land.
"""

import contextlib

import ml_dtypes
import numpy as np

import concourse.bass as bass
import concourse.bacc as bacc
import concourse.mybir as mybir
from concourse.library_config import mlp
from concourse.bass_utils import run_bass_kernel_spmd

VOCAB = 100000
E = 64                    # embedding dim; 128 B rows in bf16
N_CORES = 8
SHARD = VOCAB // N_CORES  # 12500 rows per core
UNIT = 2 * E              # 2 rows = 256 B = 128 bf16 elems (min SWDGE stride)
NUNITS = SHARD // 2       # 6250 units per core
BLK = 32                  # tokens per block descriptor
W_UNITS = 16              # window units per block (32 rows, 4 KB descriptor)
QE = W_UNITS * UNIT       # 2048 bf16 elems per block
BLK_ROWS = 2 * W_UNITS    # rows addressable inside a block window
N_PAD = 104448            # padded tokens per core (max bucket 102771 @ seed)
N_QUAD = N_PAD // BLK     # 3264 blocks
C = 1024                  # singles: indices per dma_gather (single_packet cap)
SCH = 1                   # singles chunks
F = C // 128              # singles free slots per chunk
CS = C // 16              # idx-tile columns per singles chunk
# block-chunk schedule (descriptors per dma_gather): small first chunk lets
# the write stream start early. Sum = 3328 slots (3264 blocks + padding).
SIZES = [256, 512, 512, 512, 512, 512, 512]
OFFS = [sum(SIZES[:i]) for i in range(len(SIZES))]
QCH = len(SIZES)
NSLOT = sum(SIZES)        # 3328

BF16 = ml_dtypes.bfloat16

_compiled = None


def _build():
    nc = bacc.Bacc("TRN2", num_swdge_queues=4)
    w_hbm = nc.dram_tensor("w", [NUNITS, UNIT], mybir.dt.bfloat16, kind="ExternalInput")
    qidx_hbm = nc.dram_tensor(
        "qidx", [128, NSLOT // 16], mybir.dt.int16, kind="ExternalInput"
    )
    sidx_hbm = nc.dram_tensor(
        "sidx", [128, SCH * CS], mybir.dt.int16, kind="ExternalInput"
    )
    outq_hbm = nc.dram_tensor(
        "outq", [128, (NSLOT // 128) * QE], mybir.dt.bfloat16, kind="ExternalOutput"
    )
    outs_hbm = nc.dram_tensor(
        "outs", [SCH, 128, F * UNIT], mybir.dt.bfloat16, kind="ExternalOutput"
    )

    # overlapping view of the table: "unit" u = elements [u*128, u*128 + 2048)
    w_blk = w_hbm[:].copy()
    w_blk.ap[0] = (UNIT, NUNITS - (W_UNITS - 1))
    w_blk.ap[1] = (1, QE)

    # queue / writer assignment per chunk: queue 2 -> sync writer,
    # queue 3 -> scalar writer; singles ride queue 3 / scalar.
    def q_of(k):
        return 2 + (k % 2)

    with contextlib.ExitStack() as stack:
        block = stack.enter_context(nc.Block())
        qidx_sb = stack.enter_context(
            nc.sbuf_tensor("qidx_sb", [128, NSLOT // 16], mybir.dt.int16)
        )
        sidx_sb = stack.enter_context(
            nc.sbuf_tensor("sidx_sb", [128, SCH * CS], mybir.dt.int16)
        )
        widx_sb = stack.enter_context(
            nc.sbuf_tensor("widx_sb", [128, 2], mybir.dt.int16)
        )
        wbuf = stack.enter_context(
            nc.sbuf_tensor("wbuf", [128, 1, UNIT], mybir.dt.bfloat16)
        )
        qbufs = [
            stack.enter_context(
                nc.sbuf_tensor(f"qbuf{k}", [128, SIZES[k] // 128, QE], mybir.dt.bfloat16)
            )
            for k in range(QCH)
        ]
        sbufs = [
            stack.enter_context(
                nc.sbuf_tensor(f"sbuf{k}", [128, F, UNIT], mybir.dt.bfloat16)
            )
            for k in range(SCH)
        ]
        isem = stack.enter_context(nc.semaphore("isem"))
        wsem_warm = stack.enter_context(nc.semaphore("wsem_warm"))
        # cumulative completion sems: one per SWDGE gather queue, one per
        # HWDGE writer engine
        gq = {
            2: stack.enter_context(nc.semaphore("gq2")),
            3: stack.enter_context(nc.semaphore("gq3")),
        }
        wr = {
            0: stack.enter_context(nc.semaphore("wr_sync")),
            1: stack.enter_context(nc.semaphore("wr_scal")),
        }

        # per-queue cumulative thresholds for each chunk's completion
        gthr = {}
        cnt = {2: 0, 3: 0}
        for k in range(QCH):
            cnt[q_of(k)] += 16
            gthr[k] = cnt[q_of(k)]
        sthr = cnt[3] + 16  # singles completion threshold on queue 3

        @block.sync
        def _(s: bass.BassEngine):
            # idx loads fired from sync: it comes up ~3us before gpsimd
            s.dma_start(qidx_sb[:], qidx_hbm[:]).then_inc(isem, 16)
            s.dma_start(sidx_sb[:], sidx_hbm[:]).then_inc(isem, 16)
            nwr = 0
            for k in range(0, QCH, 2):
                a = (OFFS[k] // 128) * QE
                b = ((OFFS[k] + SIZES[k]) // 128) * QE
                s.wait_ge(gq[2], gthr[k])
                s.dma_start(outq_hbm[:, a:b], qbufs[k][:]).then_inc(wr[0], 16)
                nwr += 16
            s.wait_ge(wr[0], nwr)

        @block.gpsimd
        def _(g: bass.BassGpSimd):
            g.load_library(mlp)
            # warm the DMAGatherAnt ucode while the idx loads land
            g.memzero(widx_sb[:])
            g.dma_gather(
                wbuf[:], w_hbm[:], widx_sb[:, 0:1], 16, 16, UNIT, queue_num=2
            ).then_inc(wsem_warm, 16)
            g.wait_ge(isem, 32)
            for k in range(QCH):
                sz = SIZES[k]
                g.dma_gather(
                    qbufs[k][:],
                    w_blk,
                    qidx_sb[:, OFFS[k] // 16 : (OFFS[k] + sz) // 16],
                    sz,
                    sz,
                    QE,
                    elem_step=UNIT,
                    # queues 2/3: keep gather rings off SWDGE contexts 0/1,
                    # which interleave worst with the HWDGE write rings
                    queue_num=q_of(k),
                ).then_inc(gq[q_of(k)], 16)
            for k in range(SCH):
                g.dma_gather(
                    sbufs[k][:],
                    w_hbm[:],
                    sidx_sb[:, k * CS : (k + 1) * CS],
                    C,
                    C,
                    UNIT,
                    queue_num=3,
                ).then_inc(gq[3], 16)
            g.wait_ge(wsem_warm, 16)

        @block.scalar
        def _(sc: bass.BassEngine):
            nwr = 0
            for k in range(1, QCH, 2):
                a = (OFFS[k] // 128) * QE
                b = ((OFFS[k] + SIZES[k]) // 128) * QE
                sc.wait_ge(gq[3], gthr[k])
                sc.dma_start(outq_hbm[:, a:b], qbufs[k][:]).then_inc(wr[1], 16)
                nwr += 16
            for k in range(SCH):
                sc.wait_ge(gq[3], sthr)
                sc.dma_start(outs_hbm[k], sbufs[k][:]).then_inc(wr[1], 16)
                nwr += 16
            sc.wait_ge(wr[1], nwr)

    nc.compile()
    return nc


def _get_compiled():
    global _compiled
    if _compiled is None:
        _compiled = _build()
    return _compiled


def _idx_tile(vals, nch, cs):
    """[nch*16*cs] int16 -> dma_gather layout [128, nch*cs] (i -> partition
    i%16, col chunk*cs + i//16, replicated on the 8 partition groups)."""
    t = vals.reshape(nch, cs, 16).transpose(2, 0, 1).reshape(16, -1)
    return np.tile(t, (8, 1))


def _idx_tile_sched(vals):
    """Like _idx_tile but for the tapered SIZES schedule (per-chunk wrap)."""
    cols = [
        vals[OFFS[k] : OFFS[k] + SIZES[k]].reshape(SIZES[k] // 16, 16).T
        for k in range(QCH)
    ]
    return np.tile(np.concatenate(cols, axis=1), (8, 1))


def _run(x, W, b, trace=False):
    x = np.asarray(x)
    W = np.asarray(W, dtype=np.float32)
    b = np.asarray(b, dtype=np.float32)
    orig_shape = x.shape
    xf = np.ascontiguousarray(x).reshape(-1).astype(np.int64)
    n_tok = xf.shape[0]

    # bias folded in (fp32 add, matching the reference), then bf16 round
    table = (W.T + b).astype(BF16)

    order = np.argsort(xf, kind="stable")
    counts = np.bincount(xf[order] // SHARD, minlength=N_CORES)
    starts = np.concatenate(([0], np.cumsum(counts)))[:N_CORES]

    in_maps = []
    host_jobs = []
    for c in range(N_CORES):
        n_c = int(counts[c])
        pos_c = order[starts[c] : starts[c] + n_c]
        extra_pos = None
        if n_c > N_PAD:  # statistically never; exact host fallback
            extra_pos = pos_c[N_PAD:]
            pos_c = pos_c[:N_PAD]
            n_c = N_PAD
        loc = (xf[pos_c] - c * SHARD).astype(np.int32)
        pad = np.full(N_PAD, loc[-1] if n_c else 0, dtype=np.int32)
        pad[:n_c] = loc  # tail padding keeps the array sorted

        base = np.minimum(pad[0::BLK] // 2, NUNITS - W_UNITS)
        sub = pad.reshape(-1, BLK) - 2 * base[:, None]
        ok = (sub >= 0) & (sub <= BLK_ROWS - 1)
        left_j = np.flatnonzero(~ok.reshape(-1))  # token slots needing singles
        left_j = left_j[left_j < n_c]

        # trailing pad slots get -1: skipped by the ucode (no DMA issued)
        qvals = np.full(NSLOT, -1, dtype=np.int16)
        qvals[:N_QUAD] = base.astype(np.int16)
        svals = np.full(SCH * C, -1, dtype=np.int16)
        ns = min(len(left_j), SCH * C)
        svals[:ns] = (pad[left_j[:ns]] // 2).astype(np.int16)
        svals[max(ns, 1) - 1] = max(svals[max(ns, 1) - 1], 0)  # >=1 valid
        spar = (pad[left_j[:ns]] % 2).astype(np.int64)  # row within the unit

        in_maps.append(
            {
                "w": np.ascontiguousarray(
                    table[c * SHARD : (c + 1) * SHARD].reshape(NUNITS, UNIT)
                ),
                "qidx": _idx_tile_sched(qvals),
                "sidx": _idx_tile(svals, SCH, CS),
            }
        )
        host_jobs.append((pos_c, n_c, sub, left_j, ns, spar, extra_pos))

    nc = _get_compiled()
    br = run_bass_kernel_spmd(nc, in_maps, core_ids=list(range(N_CORES)), trace=trace)

    out_full = np.empty((n_tok, E), dtype=np.float32)
    tok_quad = np.arange(N_PAD) // BLK
    for c in range(N_CORES):
        pos_c, n_c, sub, left_j, ns, spar, extra_pos = host_jobs[c]
        # block i -> [partition i%128, column (i//128)*QE] (OFFS are all
        # multiples of 128, so the per-chunk layout globalizes)
        qdev = (
            br.results[c]["outq"]
            .reshape(128, NSLOT // 128, QE)
            .transpose(1, 0, 2)
            .reshape(NSLOT, BLK_ROWS, E)
        )
        subf = np.clip(sub.reshape(-1), 0, BLK_ROWS - 1)
        rows = qdev[tok_quad, subf].astype(np.float32)  # [N_PAD, E]
        if ns:
            sdev = (
                br.results[c]["outs"]
                .reshape(SCH, 128, F, 2, E)
                .transpose(0, 2, 1, 3, 4)
                .reshape(SCH * C, 2, E)
            )
            rows[left_j[:ns]] = sdev[np.arange(ns), spar].astype(np.float32)
        if len(left_j) > ns:  # singles overflow: exact host fallback
            j = left_j[ns:]
            rows[j] = table[xf[pos_c[j]]].astype(np.float32)
        out_full[pos_c] = rows[:n_c]
        if extra_pos is not None:
            out_full[extra_pos] = table[xf[extra_pos]].astype(np.float32)

    return out_full.reshape(*orig_shape, E), br


def kernel(x, W, b):
    out, _ = _run(x, W, b, trace=False)
    return out


# revision 15
# speedup vs baseline: 8.5597x; 8.5597x over previous
"""Embedding lookup (gather rows of W.T by index, + bias) on 8 TRN2 cores.

Strategy: vocab-sharded ("row-parallel") embedding in bf16. The bias is
folded into the table on the host and the table is cast to bf16 (worst-case
elementwise relative error 2^-9 ~ 0.2%, well under the 2e-2 gate); every
device byte moved is thereby halved vs f32. Each core owns a 12500-row shard
of the 100000-row table; the host routes each token to its owning core via
one argsort (grouping by shard AND sorting ascending within it), the device
does the data movement, and the host applies the inverse permutation (and
upcast) to assemble the full [4096, 200, 64] f32 output.

Device kernel (SPMD on 8 cores, identical program), built around the gpsimd
dma_gather primitive (SWDGE: one DMA descriptor per index):

- bf16 rows are 128 B but SWDGE strides must be 256 B-aligned, so the table
  is viewed as 6250 UNITS of 2 rows (256 B). A BLOCK descriptor serves
  BLK=32 consecutive sorted tokens from a W=16-unit (32-row) window: one
  4 KB descriptor at SDMA line rate per 32 tokens. Sorted multiplicity is
  ~8.2 tokens/row, so a 32-token span covers ~4 distinct rows - the 32-row
  window essentially never misses (violators fall to the SINGLES pass).
- SINGLES pass: rare tokens outside their block window are gathered one
  256 B unit each; the host slices the right half. Unused singles slots are
  -1 (trailing negatives are skipped by the ucode), so the pass is ~free
  when empty; one valid slot is kept so the completion semaphore fires.
- 7 block chunks (<=512 descriptors each) with DEDICATED SBUF buffers - no
  buffer rotation, so gathers never wait on writes. Gathers alternate SWDGE
  queues 2/3; the two HWDGE engines (sync/scalar) stream completed buffers
  to HBM, overlapped with subsequent gathers. First chunk is small (256) to
  prime the write stream early.
- Startup: the idx loads are issued from the sync engine (up ~3 us before
  gpsimd is ready), and a dummy 16-descriptor gather warms the DMAGatherAnt
  ucode while they land.
"""

import contextlib

import ml_dtypes
import numpy as np

import concourse.bass as bass
import concourse.bacc as bacc
import concourse.mybir as mybir
from concourse.library_config import mlp
from concourse.bass_utils import run_bass_kernel_spmd

VOCAB = 100000
E = 64                    # embedding dim; 128 B rows in bf16
N_CORES = 8
SHARD = VOCAB // N_CORES  # 12500 rows per core
UNIT = 2 * E              # 2 rows = 256 B = 128 bf16 elems (min SWDGE stride)
NUNITS = SHARD // 2       # 6250 units per core
BLK = 32                  # tokens per block descriptor
W_UNITS = 16              # window units per block (32 rows, 4 KB descriptor)
QE = W_UNITS * UNIT       # 2048 bf16 elems per block
BLK_ROWS = 2 * W_UNITS    # rows addressable inside a block window
N_PAD = 104448            # padded tokens per core (max bucket 102771 @ seed)
N_QUAD = N_PAD // BLK     # 3264 blocks
C = 1024                  # singles: indices per dma_gather (single_packet cap)
SCH = 1                   # singles chunks
F = C // 128              # singles free slots per chunk
CS = C // 16              # idx-tile columns per singles chunk
# block-chunk schedule (descriptors per dma_gather): small first chunk lets
# the write stream start early. Sum = 3328 slots (3264 blocks + padding).
SIZES = [256, 512, 512, 512, 512, 512, 512]
OFFS = [sum(SIZES[:i]) for i in range(len(SIZES))]
QCH = len(SIZES)
NSLOT = sum(SIZES)        # 3328

BF16 = ml_dtypes.bfloat16

_compiled = None


def _build():
    nc = bacc.Bacc("TRN2", num_swdge_queues=4)
    w_hbm = nc.dram_tensor("w", [NUNITS, UNIT], mybir.dt.bfloat16, kind="ExternalInput")
    qidx_hbm = nc.dram_tensor(
        "qidx", [128, NSLOT // 16], mybir.dt.int16, kind="ExternalInput"
    )
    sidx_hbm = nc.dram_tensor(
        "sidx", [128, SCH * CS], mybir.dt.int16, kind="ExternalInput"
    )
    outq_hbm = nc.dram_tensor(
        "outq", [128, (NSLOT // 128) * QE], mybir.dt.bfloat16, kind="ExternalOutput"
    )
    outs_hbm = nc.dram_tensor(
        "outs", [SCH, 128, F * UNIT], mybir.dt.bfloat16, kind="ExternalOutput"
    )

    # overlapping view of the table: "unit" u = elements [u*128, u*128 + 2048)
    w_blk = w_hbm[:].copy()
    w_blk.ap[0] = (UNIT, NUNITS - (W_UNITS - 1))
    w_blk.ap[1] = (1, QE)

    # queue / writer assignment per chunk: queue 2 -> sync writer,
    # queue 3 -> scalar writer; singles ride queue 3 / scalar.
    def q_of(k):
        return 2 + (k % 2)

    with contextlib.ExitStack() as stack:
        block = stack.enter_context(nc.Block())
        qidx_sb = stack.enter_context(
            nc.sbuf_tensor("qidx_sb", [128, NSLOT // 16], mybir.dt.int16)
        )
        sidx_sb = stack.enter_context(
            nc.sbuf_tensor("sidx_sb", [128, SCH * CS], mybir.dt.int16)
        )
        widx_sb = stack.enter_context(
            nc.sbuf_tensor("widx_sb", [128, 2], mybir.dt.int16)
        )
        wbuf = stack.enter_context(
            nc.sbuf_tensor("wbuf", [128, 1, UNIT], mybir.dt.bfloat16)
        )
        qbufs = [
            stack.enter_context(
                nc.sbuf_tensor(f"qbuf{k}", [128, SIZES[k] // 128, QE], mybir.dt.bfloat16)
            )
            for k in range(QCH)
        ]
        sbufs = [
            stack.enter_context(
                nc.sbuf_tensor(f"sbuf{k}", [128, F, UNIT], mybir.dt.bfloat16)
            )
            for k in range(SCH)
        ]
        isem = stack.enter_context(nc.semaphore("isem"))
        wsem_warm = stack.enter_context(nc.semaphore("wsem_warm"))
        # cumulative completion sems: one per SWDGE gather queue, one per
        # HWDGE writer engine
        gq = {
            2: stack.enter_context(nc.semaphore("gq2")),
            3: stack.enter_context(nc.semaphore("gq3")),
        }
        wr = {
            0: stack.enter_context(nc.semaphore("wr_sync")),
            1: stack.enter_context(nc.semaphore("wr_scal")),
        }

        # per-queue cumulative thresholds for each chunk's completion
        gthr = {}
        cnt = {2: 0, 3: 0}
        for k in range(QCH):
            cnt[q_of(k)] += 16
            gthr[k] = cnt[q_of(k)]
        sthr = cnt[3] + 16  # singles completion threshold on queue 3

        @block.sync
        def _(s: bass.BassEngine):
            # idx loads fired from sync: it comes up ~3us before gpsimd
            s.dma_start(qidx_sb[:], qidx_hbm[:]).then_inc(isem, 16)
            s.dma_start(sidx_sb[:], sidx_hbm[:]).then_inc(isem, 16)
            nwr = 0
            for k in range(0, QCH, 2):
                a = (OFFS[k] // 128) * QE
                b = ((OFFS[k] + SIZES[k]) // 128) * QE
                s.wait_ge(gq[2], gthr[k])
                s.dma_start(outq_hbm[:, a:b], qbufs[k][:]).then_inc(wr[0], 16)
                nwr += 16
            s.wait_ge(wr[0], nwr)

        @block.gpsimd
        def _(g: bass.BassGpSimd):
            g.load_library(mlp)
            # warm the DMAGatherAnt ucode while the idx loads land
            g.memzero(widx_sb[:])
            g.dma_gather(
                wbuf[:], w_hbm[:], widx_sb[:, 0:1], 16, 16, UNIT, queue_num=2
            ).then_inc(wsem_warm, 16)
            g.wait_ge(isem, 32)
            for k in range(QCH):
                sz = SIZES[k]
                g.dma_gather(
                    qbufs[k][:],
                    w_blk,
                    qidx_sb[:, OFFS[k] // 16 : (OFFS[k] + sz) // 16],
                    sz,
                    sz,
                    QE,
                    elem_step=UNIT,
                    # queues 2/3: keep gather rings off SWDGE contexts 0/1,
                    # which interleave worst with the HWDGE write rings
                    queue_num=q_of(k),
                ).then_inc(gq[q_of(k)], 16)
            for k in range(SCH):
                g.dma_gather(
                    sbufs[k][:],
                    w_hbm[:],
                    sidx_sb[:, k * CS : (k + 1) * CS],
                    C,
                    C,
                    UNIT,
                    queue_num=3,
                ).then_inc(gq[3], 16)
            g.wait_ge(wsem_warm, 16)

        @block.scalar
        def _(sc: bass.BassEngine):
            nwr = 0
            for k in range(1, QCH, 2):
                a = (OFFS[k] // 128) * QE
                b = ((OFFS[k] + SIZES[k]) // 128) * QE
                sc.wait_ge(gq[3], gthr[k])
                sc.dma_start(outq_hbm[:, a:b], qbufs[k][:]).then_inc(wr[1], 16)
                nwr += 16
            for k in range(SCH):
                sc.wait_ge(gq[3], sthr)
                sc.dma_start(outs_hbm[k], sbufs[k][:]).then_inc(wr[1], 16)
                nwr += 16
            sc.wait_ge(wr[1], nwr)

    nc.compile()
    return nc


def _get_compiled():
    global _compiled
    if _compiled is None:
        _compiled = _build()
    return _compiled


def _idx_tile(vals, nch, cs):
    """[nch*16*cs] int16 -> dma_gather layout [128, nch*cs] (i -> partition
    i%16, col chunk*cs + i//16, replicated on the 8 partition groups)."""
    t = vals.reshape(nch, cs, 16).transpose(2, 0, 1).reshape(16, -1)
    return np.tile(t, (8, 1))


def _idx_tile_sched(vals):
    """Like _idx_tile but for the tapered SIZES schedule (per-chunk wrap)."""
    cols = [
        vals[OFFS[k] : OFFS[k] + SIZES[k]].reshape(SIZES[k] // 16, 16).T
        for k in range(QCH)
    ]
    return np.tile(np.concatenate(cols, axis=1), (8, 1))


def _run(x, W, b, trace=False):
    x = np.asarray(x)
    W = np.asarray(W, dtype=np.float32)
    b = np.asarray(b, dtype=np.float32)
    orig_shape = x.shape
    xf = np.ascontiguousarray(x).reshape(-1).astype(np.int64)
    n_tok = xf.shape[0]

    # bias folded in (fp32 add, matching the reference), then bf16 round
    table = (W.T + b).astype(BF16)

    order = np.argsort(xf, kind="stable")
    counts = np.bincount(xf[order] // SHARD, minlength=N_CORES)
    starts = np.concatenate(([0], np.cumsum(counts)))[:N_CORES]

    in_maps = []
    host_jobs = []
    for c in range(N_CORES):
        n_c = int(counts[c])
        pos_c = order[starts[c] : starts[c] + n_c]
        extra_pos = None
        if n_c > N_PAD:  # statistically never; exact host fallback
            extra_pos = pos_c[N_PAD:]
            pos_c = pos_c[:N_PAD]
            n_c = N_PAD
        loc = (xf[pos_c] - c * SHARD).astype(np.int32)
        pad = np.full(N_PAD, loc[-1] if n_c else 0, dtype=np.int32)
        pad[:n_c] = loc  # tail padding keeps the array sorted

        base = np.minimum(pad[0::BLK] // 2, NUNITS - W_UNITS)
        sub = pad.reshape(-1, BLK) - 2 * base[:, None]
        ok = (sub >= 0) & (sub <= BLK_ROWS - 1)
        left_j = np.flatnonzero(~ok.reshape(-1))  # token slots needing singles
        left_j = left_j[left_j < n_c]

        # trailing pad slots get -1: skipped by the ucode (no DMA issued)
        qvals = np.full(NSLOT, -1, dtype=np.int16)
        qvals[:N_QUAD] = base.astype(np.int16)
        svals = np.full(SCH * C, -1, dtype=np.int16)
        ns = min(len(left_j), SCH * C)
        svals[:ns] = (pad[left_j[:ns]] // 2).astype(np.int16)
        svals[max(ns, 1) - 1] = max(svals[max(ns, 1) - 1], 0)  # >=1 valid
        spar = (pad[left_j[:ns]] % 2).astype(np.int64)  # row within the unit

        in_maps.append(
            {
                "w": np.ascontiguousarray(
                    table[c * SHARD : (c + 1) * SHARD].reshape(NUNITS, UNIT)
                ),
                "qidx": _idx_tile_sched(qvals),
                "sidx": _idx_tile(svals, SCH, CS),
            }
        )
        host_jobs.append((pos_c, n_c, sub, left_j, ns, spar, extra_pos))

    nc = _get_compiled()
    br = run_bass_kernel_spmd(nc, in_maps, core_ids=list(range(N_CORES)), trace=trace)

    out_full = np.empty((n_tok, E), dtype=np.float32)
    tok_quad = np.arange(N_PAD) // BLK
    for c in range(N_CORES):
        pos_c, n_c, sub, left_j, ns, spar, extra_pos = host_jobs[c]
        # block i -> [partition i%128, column (i//128)*QE] (OFFS are all
        # multiples of 128, so the per-chunk layout globalizes)
        qdev = (
            br.results[c]["outq"]
            .reshape(128, NSLOT // 128, QE)
            .transpose(1, 0, 2)
            .reshape(NSLOT, BLK_ROWS, E)
        )
        subf = np.clip(sub.reshape(-1), 0, BLK_ROWS - 1)
        rows = qdev[tok_quad, subf].astype(np.float32)  # [N_PAD, E]
        if ns:
            sdev = (
                br.results[c]["outs"]
                .reshape(SCH, 128, F, 2, E)
                .transpose(0, 2, 1, 3, 4)
                .reshape(SCH * C, 2, E)
            )
            rows[left_j[:ns]] = sdev[np.arange(ns), spar].astype(np.float32)
        if len(left_j) > ns:  # singles overflow: exact host fallback
            j = left_j[ns:]
            rows[j] = table[xf[pos_c[j]]].astype(np.float32)
        out_full[pos_c] = rows[:n_c]
        if extra_pos is not None:
            out_full[extra_pos] = table[xf[extra_pos]].astype(np.float32)

    return out_full.reshape(*orig_shape, E), br


def kernel(x, W, b):
    out, _ = _run(x, W, b, trace=False)
    return out
